# revision 7
# baseline (speedup 1.0000x reference)
"""Trainium2 Bass kernel for the nn_Decoder problem.

Contract: kernel(**inputs) takes the FULL (unsharded) numpy inputs and
returns the FULL output [4, 1024, 1024] f32.

Sharding: (batch b, sequence-half h) -> 8 cores, zero collectives.
Key structural fact about the reference: inside the depth loop, x is
overwritten by `_ln(_attn(x0,x0,x0,a1) + x0, ...)` which does not read
the previous iteration's x, so only the LAST layer's block contributes
to the output. The kernel therefore computes a single layer (params[-1]).

SPMD trick: the program is identical on all cores; the host rotates the
x0^T key columns per core so that the core's own 512 query tokens are
always columns 0..511 (attention is permutation-invariant over keys).
"""

import os
import sys
from contextlib import ExitStack

for _p in ("/opt/trn_rl_repo", "/root/.axon_site/_ro/trn_rl_repo"):
    if os.path.isdir(_p) and _p not in sys.path:
        sys.path.insert(0, _p)

import numpy as np
import ml_dtypes

import concourse.bass as bass
import concourse.tile as tile
from concourse import bacc, mybir
from concourse.bass import ds, ts
from concourse.bass_utils import run_bass_kernel_spmd

BF16 = mybir.dt.bfloat16
F32 = mybir.dt.float32
AF = mybir.ActivationFunctionType
OP = mybir.AluOpType

P = 128
DIM = 1024          # model dim (8 tiles of 128)
HEADS = 16
DH = 64             # head dim
INNER = HEADS * DH  # 1024
MLP = 4096          # 32 tiles of 128, 8 column-blocks of 512
T = 1024            # full decoder sequence (8 key tiles)
TQ = 512            # tokens per core (4 tile rows)
S = 512             # encoder sequence (4 key tiles)
SCALE = DH ** -0.5
EPS = 1e-5
NC = DIM // P       # 8  contraction tiles over model dim
NI = INNER // P     # 8  inner-dim tiles (2 heads each)
NKT = T // P        # 8  self-attn key tiles
NKS = S // P        # 4  cross-attn key tiles
NMT = TQ // P       # 4  own-token tiles
VW = HEADS * (DH + 1)  # 1040: V tiles carry a ones column per head


def _bcast_row_ap(row_ap, nparts):
    """Broadcast a [1, N] SBUF row for DMA: keep the (nonzero-step)
    partition dim at count 1 and replicate via a 0-step free dim."""
    return bass.AP(
        tensor=row_ap.tensor,
        offset=row_ap.offset,
        ap=[list(row_ap.ap[0]), [0, nparts], list(row_ap.ap[-1])],
    )


def build_program(flags, sim_gelu=False):
    """Build the single-core SPMD program. flags: dict of bias/affine gates."""
    nc = bacc.Bacc("TRN2", target_bir_lowering=False, debug=False)

    # ---- DRAM I/O ----------------------------------------------------
    d_x0T = nc.dram_tensor("x0T", [DIM, T], BF16, kind="ExternalInput")
    d_x0h = nc.dram_tensor("x0h", [TQ, DIM], F32, kind="ExternalInput")
    d_encT = nc.dram_tensor("encT", [DIM, S], BF16, kind="ExternalInput")
    d_mask = nc.dram_tensor("maskb", [P, NKS], F32, kind="ExternalInput")
    dw = {}
    for nm in ("wq1", "wk1", "wv1", "wo1", "wq2", "wk2", "wv2", "wo2"):
        dw[nm] = nc.dram_tensor(nm, [DIM, DIM], BF16, kind="ExternalInput")
    dw["w1"] = nc.dram_tensor("w1", [DIM, MLP], BF16, kind="ExternalInput")
    dw["w2"] = nc.dram_tensor("w2", [MLP, DIM], BF16, kind="ExternalInput")
    if flags["b1"]:
        d_b1 = nc.dram_tensor("b1pk", [P, MLP // P], F32, kind="ExternalInput")
    d_rep = {}
    for nm in ("bo1", "bo2", "b2", "g1", "bb1", "g2", "bb2", "g3", "bb3"):
        if flags[nm]:
            d_rep[nm] = nc.dram_tensor("rep_" + nm, [P, DIM], F32,
                                       kind="ExternalInput")
    d_out = nc.dram_tensor("outp", [TQ, DIM], F32, kind="ExternalOutput")

    with tile.TileContext(nc) as tc, ExitStack() as ctx:
        # ---- whole-kernel pools (per-partition KB is the scarce axis) -
        persist = ctx.enter_context(tc.tile_pool(name="persist", bufs=1))
        xp = ctx.enter_context(tc.tile_pool(name="xp", bufs=8))
        xbp = ctx.enter_context(tc.tile_pool(name="xbp", bufs=2))
        xtp = ctx.enter_context(tc.tile_pool(name="xtp", bufs=9))
        sm = ctx.enter_context(tc.tile_pool(name="sm", bufs=2))
        stat = ctx.enter_context(tc.tile_pool(name="stat", bufs=6))
        ps_p = ctx.enter_context(tc.tile_pool(name="ps_p", bufs=3, space="PSUM"))

        # ---- persistent loads ---------------------------------------
        x0T_sb = persist.tile([P, NC, T], BF16)
        nc.sync.dma_start(x0T_sb, d_x0T.ap().rearrange("(o p) f -> p o f", p=P))
        encT_sb = persist.tile([P, NC, S], BF16)
        nc.sync.dma_start(encT_sb, d_encT.ap().rearrange("(o p) f -> p o f", p=P))
        mask_sb = persist.tile([P, NKS], F32)
        nc.sync.dma_start(mask_sb, d_mask.ap())
        eps_sb = persist.tile([P, 1], F32)
        nc.vector.memset(eps_sb, EPS)
        if flags["b1"]:
            b1_sb = persist.tile([P, MLP // P], F32)
            nc.sync.dma_start(b1_sb, d_b1.ap())
        rep_sb = {}
        for nm, dt_ in d_rep.items():
            rep_sb[nm] = persist.tile([P, DIM], F32, name="rep_" + nm)
            nc.sync.dma_start(rep_sb[nm], dt_.ap())

        def layer_norm(xr, g_key, b_key):
            st = stat.tile([P, 2, 6], F32, tag="bnst")
            nc.vector.bn_stats(st[:, 0, :], xr[:, 0:512])
            nc.vector.bn_stats(st[:, 1, :], xr[:, 512:1024])
            mv = stat.tile([P, 2], F32, tag="bnmv")
            nc.vector.bn_aggr(mv, st)
            rs = stat.tile([P, 1], F32, tag="rstd")
            nc.scalar.activation(rs, mv[:, 1:2], AF.Sqrt, bias=eps_sb)
            nc.vector.reciprocal(rs, rs)
            nc.vector.tensor_scalar(xr, xr, mv[:, 0:1], rs,
                                    op0=OP.subtract, op1=OP.mult)
            if flags[g_key]:
                nc.vector.tensor_mul(xr, xr, rep_sb[g_key])
            if flags[b_key]:
                nc.vector.tensor_add(xr, xr, rep_sb[b_key])

        x2f = x2T = None
        with ExitStack() as actx:
            wa = actx.enter_context(tc.tile_pool(name="wa", bufs=2))
            qtp = actx.enter_context(tc.tile_pool(name="qtp", bufs=17))
            ktp = actx.enter_context(tc.tile_pool(name="ktp", bufs=8))
            vp = actx.enter_context(tc.tile_pool(name="vp", bufs=9))
            sxp = actx.enter_context(tc.tile_pool(name="sxp", bufs=10))
            ocp = actx.enter_context(tc.tile_pool(name="ocp", bufs=9))
            ps_s = actx.enter_context(
                tc.tile_pool(name="ps_s", bufs=2, space="PSUM"))
            ps_av = actx.enter_context(
                tc.tile_pool(name="ps_av", bufs=2, space="PSUM"))

            # attn weights stream as half-matrices [P, 4, DIM] (8KB/part)
            def load_w(name):
                halves = []
                for hh in range(2):
                    w = wa.tile([P, NC // 2, DIM], BF16, tag="wattn",
                                name=f"{name}_{hh}")
                    nc.sync.dma_start(
                        w, dw[name].ap().rearrange("(o p) f -> p o f", p=P)
                        [:, ds(4 * hh, 4), :])
                    halves.append(w)
                return lambda c: halves[c // 4][:, c % 4, :]

            def projT(wsel, rhs_fn, nfree, tag):
                outs = []
                for i in range(NI):
                    ps = ps_p.tile([P, nfree], F32, tag="pp", name=f"ps_{tag}{i}")
                    for c in range(NC):
                        nc.tensor.matmul(ps, wsel(c)[:, ts(i, P)], rhs_fn(c),
                                         start=(c == 0), stop=(c == NC - 1))
                    o = qtp.tile([P, nfree], BF16, tag="qt", name=f"{tag}{i}")
                    nc.scalar.activation(o, ps, AF.Copy)
                    outs.append(o)
                return outs

            def make_v(wsel, lhsT_fn, nkt, tag):
                vt = []
                for kt in range(nkt):
                    v = vp.tile([P, VW], BF16, tag="v", name=f"{tag}{kt}")
                    ones_ap = v.rearrange("p (h c) -> p h c", c=DH + 1)[
                        :, :, DH:DH + 1]
                    nc.vector.memset(ones_ap, 1.0)
                    for nch in range(2):
                        ps = ps_p.tile([P, 512], F32, tag="pp",
                                       name=f"ps_{tag}{kt}_{nch}")
                        for c in range(NC):
                            nc.tensor.matmul(ps, lhsT_fn(c, kt),
                                             wsel(c)[:, ds(512 * nch, 512)],
                                             start=(c == 0), stop=(c == NC - 1))
                        dst = v.rearrange("p (h c) -> p h c", c=DH + 1)[
                            :, 8 * nch:8 * nch + 8, 0:DH]
                        src = ps.rearrange("p (h c) -> p h c", c=DH)
                        nc.vector.tensor_copy(dst, src)
                    vt.append(v)
                return vt

            def attn_heads(QTt, KTt, Vt, nkt, masked, tag):
                ocat = [ocp.tile([P, TQ], BF16, tag="oc", name=f"{tag}_{i}")
                        for i in range(NI)]
                for h in range(HEADS):
                    i, b0 = h // 2, DH * (h % 2)
                    exps = []
                    for kt in range(nkt):
                        pss = ps_s.tile([P, TQ], F32, tag="ps",
                                        name=f"pss_{tag}{h}_{kt}")
                        nc.tensor.matmul(pss, KTt[i][b0:b0 + DH, ts(kt, P)],
                                         QTt[i][b0:b0 + DH, :],
                                         start=True, stop=True)
                        e = sxp.tile([P, TQ], BF16, tag="ex",
                                     name=f"ex_{tag}{h}_{kt}")
                        if masked:
                            nc.scalar.activation(e, pss, AF.Exp, scale=SCALE,
                                                 bias=mask_sb[:, kt:kt + 1])
                        else:
                            nc.scalar.activation(e, pss, AF.Exp, scale=SCALE)
                        exps.append(e)
                    pav = ps_av.tile([DH + 1, TQ], F32, tag="pav",
                                     name=f"pav_{tag}{h}")
                    for kt in range(nkt):
                        nc.tensor.matmul(pav,
                                         Vt[kt][:, ds((DH + 1) * h, DH + 1)],
                                         exps[kt], start=(kt == 0),
                                         stop=(kt == nkt - 1))
                    rt = sm.tile([DH + 1, TQ], F32, tag="recip",
                                 name=f"rt_{tag}{h}")
                    nc.vector.reciprocal(rt[DH:DH + 1, :], pav[DH:DH + 1, :])
                    bt = sm.tile([DH, TQ], F32, tag="bcast",
                                 name=f"bt_{tag}{h}")
                    nc.gpsimd.dma_start(bt, _bcast_row_ap(rt[DH:DH + 1, :], DH))
                    if h % 2 == 0:
                        nc.vector.tensor_mul(ocat[i][0:DH, :], pav[0:DH, :], bt)
                    else:
                        ot = sm.tile([DH, TQ], BF16, tag="otmp",
                                     name=f"ot_{tag}{h}")
                        nc.vector.tensor_mul(ot, pav[0:DH, :], bt)
                        nc.gpsimd.dma_start(ocat[i][DH:P, :], ot)
                return ocat

            def outproj_res_ln(ocat, wsel, res_tiles, bias_key, g_key, b_key):
                xres = []
                xT = [xtp.tile([P, TQ], BF16, tag="xT", name=f"xT{b_key}_{j}")
                      for j in range(NC)]
                for mt in range(NMT):
                    xr = xp.tile([P, DIM], F32, tag="xr", name=f"xr{b_key}_{mt}")
                    if res_tiles is None:
                        res = xp.tile([P, DIM], F32, tag="x0r", bufs=2,
                                      name=f"x0r_{mt}")
                        nc.sync.dma_start(res, d_x0h.ap()[ds(P * mt, P), :])
                    else:
                        res = res_tiles[mt]
                    for nch in range(2):
                        ps = ps_p.tile([P, 512], F32, tag="pp",
                                       name=f"psop{b_key}_{mt}_{nch}")
                        for i in range(NI):
                            nc.tensor.matmul(ps, ocat[i][:, ts(mt, P)],
                                             wsel(i)[:, ds(512 * nch, 512)],
                                             start=(i == 0), stop=(i == NI - 1))
                        nc.vector.tensor_add(xr[:, ds(512 * nch, 512)], ps,
                                             res[:, ds(512 * nch, 512)])
                    if flags[bias_key]:
                        nc.vector.tensor_add(xr, xr, rep_sb[bias_key])
                    layer_norm(xr, g_key, b_key)
                    xb = xbp.tile([P, DIM], BF16, tag="xb", name=f"xb{b_key}_{mt}")
                    nc.vector.tensor_copy(xb, xr)
                    for j in range(NC):
                        nc.sync.dma_start_transpose(xT[j][:, ts(mt, P)],
                                                    xb[:, ts(j, P)])
                    xres.append(xr)
                return xres, xT

            # ---- attn1 projections (queries are key-columns 0..TQ) --
            wq1 = load_w("wq1")
            QT1 = projT(wq1, lambda c: x0T_sb[:, c, 0:TQ], TQ, "qt1")
            wk1 = load_w("wk1")
            KT1 = []
            for i in range(NI):
                k = ktp.tile([P, T], BF16, tag="kt1", name=f"kt1_{i}")
                for nch in range(2):
                    ps = ps_p.tile([P, 512], F32, tag="pp",
                                   name=f"psk1_{i}_{nch}")
                    for c in range(NC):
                        nc.tensor.matmul(ps, wk1(c)[:, ts(i, P)],
                                         x0T_sb[:, c, ds(512 * nch, 512)],
                                         start=(c == 0), stop=(c == NC - 1))
                    nc.scalar.activation(k[:, ds(512 * nch, 512)], ps, AF.Copy)
                KT1.append(k)
            wv1 = load_w("wv1")
            V1 = make_v(wv1, lambda c, kt: x0T_sb[:, c, ts(kt, P)], NKT, "v1")
            # cross-attn K ready early: PE filler while ACT runs exp
            wk2 = load_w("wk2")
            K2T = projT(wk2, lambda c: encT_sb[:, c, :], S, "k2t")

            # ---- attn1 heads, out-proj, LN1 -------------------------
            Ocat1 = attn_heads(QT1, KT1, V1, NKT, masked=False, tag="oc1")
            wo1 = load_w("wo1")
            x1f, x1T = outproj_res_ln(Ocat1, wo1, None, "bo1", "g1", "bb1")

            # cross-attn V (fills the LN1/transpose gap)
            wv2 = load_w("wv2")
            V2 = make_v(wv2, lambda c, kt: encT_sb[:, c, ts(kt, P)], NKS, "v2")

            # ---- attn2 ----------------------------------------------
            wq2 = load_w("wq2")
            QT2 = projT(wq2, lambda c: x1T[c], TQ, "qt2")
            Ocat2 = attn_heads(QT2, K2T, V2, NKS, masked=True, tag="oc2")
            wo2 = load_w("wo2")
            x2f, x2T = outproj_res_ln(Ocat2, wo2, x1f, "bo2", "g2", "bb2")

        # ---- FFN ----------------------------------------------------
        with ExitStack() as fctx:
            ffw1 = fctx.enter_context(tc.tile_pool(name="ffw1", bufs=2))
            ffw2 = fctx.enter_context(tc.tile_pool(name="ffw2", bufs=6))
            htp = fctx.enter_context(tc.tile_pool(name="htp", bufs=32))
            ps_f = fctx.enter_context(
                tc.tile_pool(name="ps_f", bufs=4, space="PSUM"))

            w1r = dw["w1"].ap().rearrange("(o p) f -> p o f", p=P)
            hT = []
            for g in range(8):
                wcb = ffw1.tile([P, NC, 512], BF16, tag="w1cb", name=f"w1cb{g}")
                nc.sync.dma_start(wcb, w1r[:, :, ds(512 * g, 512)])
                for m4 in range(4):
                    ps = ps_p.tile([P, TQ], F32, tag="pp", name=f"psh{g}_{m4}")
                    for c in range(NC):
                        nc.tensor.matmul(ps, wcb[:, c, ts(m4, P)], x2T[c],
                                         start=(c == 0), stop=(c == NC - 1))
                    ht = htp.tile([P, TQ], BF16, tag="ht", name=f"ht{g}_{m4}")
                    b1b = (b1_sb[:, 4 * g + m4:4 * g + m4 + 1]
                           if flags["b1"] else 0.0)
                    if sim_gelu:
                        # CoreSim lacks Gelu; sigmoid-approx for sim runs only
                        sg = htp.tile([P, TQ], F32, tag="sg", bufs=2,
                                      name=f"sg{g}_{m4}")
                        nc.scalar.activation(sg, ps, AF.Sigmoid, scale=1.702,
                                             bias=b1b)
                        nc.vector.tensor_mul(ht, ps, sg)
                    else:
                        nc.scalar.activation(ht, ps, AF.Gelu, bias=b1b)
                    hT.append(ht)

            x3r = [xp.tile([P, DIM], F32, tag="xr", name=f"x3r_{mt}")
                   for mt in range(NMT)]
            for nch in range(2):
                psy = [ps_f.tile([P, 512], F32, tag="pf", name=f"psy{nch}_{mt}")
                       for mt in range(NMT)]
                for kt in range(MLP // P):
                    w2s = ffw2.tile([P, DIM], BF16, tag="w2s",
                                    name=f"w2s{nch}_{kt}")
                    nc.sync.dma_start(w2s, dw["w2"].ap()[ds(P * kt, P), :])
                    for mt in range(NMT):
                        nc.tensor.matmul(psy[mt], hT[kt][:, ts(mt, P)],
                                         w2s[:, ds(512 * nch, 512)],
                                         start=(kt == 0),
                                         stop=(kt == MLP // P - 1))
                for mt in range(NMT):
                    nc.vector.tensor_add(x3r[mt][:, ds(512 * nch, 512)],
                                         psy[mt],
                                         x2f[mt][:, ds(512 * nch, 512)])
            for mt in range(NMT):
                if flags["b2"]:
                    nc.vector.tensor_add(x3r[mt], x3r[mt], rep_sb["b2"])
                layer_norm(x3r[mt], "g3", "bb3")
                nc.sync.dma_start(d_out.ap()[ds(P * mt, P), :], x3r[mt])

    nc.compile()
    return nc


def _host_prep(inputs):
    """Numpy-side sharding and packing. Returns (flags, in_maps)."""
    f32 = np.float32
    bf16 = ml_dtypes.bfloat16
    dec = np.asarray(inputs["dec_input"], f32)        # [4, 1024]
    enc = np.asarray(inputs["enc_output"], f32)       # [4, 512, 1024]
    enc_in = np.asarray(inputs["enc_input"], f32)     # [4, 511]
    pe_w = np.asarray(inputs["pe_w"], f32)
    pe_b = np.asarray(inputs["pe_b"], f32)
    pos = np.asarray(inputs["pos_emb"], f32)
    lp = inputs["params"][-1]                         # only the last layer matters
    a1, a2, ff = lp["a1"], lp["a2"], lp["ff"]

    W = {
        "wq1": a1["wq"], "wk1": a1["wk"], "wv1": a1["wv"], "wo1": a1["wo"],
        "wq2": a2["wq"], "wk2": a2["wk"], "wv2": a2["wv"], "wo2": a2["wo"],
        "w1": ff["w1"], "w2": ff["w2"],
    }
    W = {k: np.ascontiguousarray(np.asarray(v, f32).astype(bf16))
         for k, v in W.items()}

    def vec(x):
        return np.asarray(x, f32).reshape(-1)

    bo1, bo2 = vec(a1["bo"]), vec(a2["bo"])
    b1v, b2v = vec(ff["b1"]), vec(ff["b2"])
    g1, bb1 = vec(a1["g"]), vec(a1["b"])
    g2, bb2 = vec(a2["g"]), vec(a2["b"])
    g3, bb3 = vec(ff["g"]), vec(ff["b"])
    flags = {
        "bo1": bool(np.any(bo1 != 0)), "bo2": bool(np.any(bo2 != 0)),
        "b1": bool(np.any(b1v != 0)), "b2": bool(np.any(b2v != 0)),
        "g1": bool(np.any(g1 != 1)), "bb1": bool(np.any(bb1 != 0)),
        "g2": bool(np.any(g2 != 1)), "bb2": bool(np.any(bb2 != 0)),
        "g3": bool(np.any(g3 != 1)), "bb3": bool(np.any(bb3 != 0)),
    }

    shared = dict(W)
    if flags["b1"]:
        shared["b1pk"] = np.ascontiguousarray(
            b1v.reshape(MLP // P, P).T.astype(f32))
    reps = {"bo1": bo1, "bo2": bo2, "b2": b2v, "g1": g1, "bb1": bb1,
            "g2": g2, "bb2": bb2, "g3": g3, "bb3": bb3}
    for nm, v in reps.items():
        if flags[nm]:
            shared["rep_" + nm] = np.ascontiguousarray(
                np.broadcast_to(v[None, :], (P, DIM)).astype(f32))

    # x0 = dec[..., None] * pe_w + pe_b + pos  (matches reference f32 order)
    x0 = (dec[..., None] * pe_w + pe_b) + pos         # [4, 1024, 1024]
    x0 = np.asarray(x0, f32)

    # cross-attn additive mask bias per batch: keys [1 | enc_input]
    enc_cat = np.concatenate([np.ones((4, 1), f32), enc_in], axis=1)  # [4,512]
    mbias = np.where(enc_cat == 0.0, f32(-1e9), f32(0.0)).astype(f32)

    in_maps = []
    for core in range(8):
        b, h = core // 2, core % 2
        x0b = x0[b]                                   # [1024, 1024]
        # rotate tokens so own queries are key-columns 0..TQ-1
        order = np.r_[h * TQ:(h + 1) * TQ, (1 - h) * TQ:(2 - h) * TQ]
        x0Tb = np.ascontiguousarray(x0b[order].T.astype(bf16))
        m = {
            "x0T": x0Tb,
            "x0h": np.ascontiguousarray(x0b[h * TQ:(h + 1) * TQ]),
            "encT": np.ascontiguousarray(enc[b].T.astype(bf16)),
            "maskb": np.ascontiguousarray(mbias[b].reshape(NKS, P).T),
        }
        m.update(shared)
        in_maps.append(m)
    return flags, in_maps


_CACHE = {}
LAST_RESULTS = None


def kernel(**inputs) -> np.ndarray:
    global LAST_RESULTS
    flags, in_maps = _host_prep(inputs)
    key = tuple(sorted(flags.items()))
    if key not in _CACHE:
        _CACHE[key] = build_program(flags)
    nc = _CACHE[key]
    res = run_bass_kernel_spmd(nc, in_maps, core_ids=list(range(8)))
    LAST_RESULTS = res
    out = np.empty((4, T, DIM), np.float32)
    for core in range(8):
        b, h = core // 2, core % 2
        out[b, h * TQ:(h + 1) * TQ, :] = res.results[core]["outp"]
    return out


# revision 28
# speedup vs baseline: 1.1502x; 1.1502x over previous
"""Trainium2 Bass kernel for the nn_Decoder problem.

Contract: kernel(**inputs) takes the FULL (unsharded) numpy inputs and
returns the FULL output [4, 1024, 1024] f32.

Sharding: (batch b, sequence-half h) -> 8 cores, zero collectives.
Key structural fact about the reference: inside the depth loop, x is
overwritten by `_ln(_attn(x0,x0,x0,a1) + x0, ...)` which does not read
the previous iteration's x, so only the LAST layer's block contributes
to the output. The kernel therefore computes a single layer (params[-1]).

SPMD trick: the program is identical on all cores; the host rotates the
x0^T key columns per core so that the core's own 512 query tokens are
always columns 0..511 (attention is permutation-invariant over keys).
"""

import os
import sys
from contextlib import ExitStack

for _p in ("/opt/trn_rl_repo", "/root/.axon_site/_ro/trn_rl_repo"):
    if os.path.isdir(_p) and _p not in sys.path:
        sys.path.insert(0, _p)

import numpy as np
import ml_dtypes

import concourse.bass as bass
import concourse.tile as tile
from concourse import bacc, mybir
from concourse.bass import ds, ts
from concourse.bass_utils import run_bass_kernel_spmd

BF16 = mybir.dt.bfloat16
F32 = mybir.dt.float32
AF = mybir.ActivationFunctionType
OP = mybir.AluOpType

P = 128
DIM = 1024          # model dim (8 tiles of 128)
HEADS = 16
DH = 64             # head dim
INNER = HEADS * DH  # 1024
MLP = 4096          # 32 tiles of 128, 8 column-blocks of 512
T = 1024            # full decoder sequence (8 key tiles)
TQ = 512            # tokens per core (4 tile rows)
S = 512             # encoder sequence (4 key tiles)
SCALE = DH ** -0.5
EPS = 1e-5
NC = DIM // P       # 8  contraction tiles over model dim
NI = INNER // P     # 8  inner-dim tiles (2 heads each)
NKT = T // P        # 8  self-attn key tiles
NKS = S // P        # 4  cross-attn key tiles
NMT = TQ // P       # 4  own-token tiles
VW = HEADS * (DH + 1)  # 1040: V tiles carry a ones column per head


def _bcast_row_ap(row_ap, nparts):
    """Broadcast a [1, N] SBUF row for DMA: keep the (nonzero-step)
    partition dim at count 1 and replicate via a 0-step free dim."""
    return bass.AP(
        tensor=row_ap.tensor,
        offset=row_ap.offset,
        ap=[list(row_ap.ap[0]), [0, nparts], list(row_ap.ap[-1])],
    )


def build_program(flags, sim_gelu=False):
    """Build the single-core SPMD program. flags: dict of bias/affine gates."""
    nc = bacc.Bacc("TRN2", target_bir_lowering=False, debug=False)

    # ---- DRAM I/O ----------------------------------------------------
    d_x0T = nc.dram_tensor("x0T", [DIM, T], BF16, kind="ExternalInput")
    d_x0h = nc.dram_tensor("x0h", [TQ, DIM], F32, kind="ExternalInput")
    d_encT = nc.dram_tensor("encT", [DIM, S], BF16, kind="ExternalInput")
    d_mask = nc.dram_tensor("maskb", [P, NKS], F32, kind="ExternalInput")
    dw = {}
    for nm in ("wq1", "wk1", "wv1", "wo1", "wq2", "wk2", "wv2", "wo2"):
        dw[nm] = nc.dram_tensor(nm, [DIM, DIM], BF16, kind="ExternalInput")
    dw["w1"] = nc.dram_tensor("w1", [DIM, MLP], BF16, kind="ExternalInput")
    dw["w2"] = nc.dram_tensor("w2", [MLP, DIM], BF16, kind="ExternalInput")
    if flags["b1"]:
        d_b1 = nc.dram_tensor("b1pk", [P, MLP // P], F32, kind="ExternalInput")
    d_rep = {}
    for nm in ("bo1", "bo2", "b2", "g1", "bb1", "g2", "bb2", "g3", "bb3"):
        if flags[nm]:
            d_rep[nm] = nc.dram_tensor("rep_" + nm, [P, DIM], F32,
                                       kind="ExternalInput")
    d_out = nc.dram_tensor("outp", [TQ, DIM], F32, kind="ExternalOutput")

    with tile.TileContext(nc) as tc, ExitStack() as ctx:
        # ---- whole-kernel pools (per-partition KB is the scarce axis) -
        persist = ctx.enter_context(tc.tile_pool(name="persist", bufs=1))
        xp = ctx.enter_context(tc.tile_pool(name="xp", bufs=8))
        xbp = ctx.enter_context(tc.tile_pool(name="xbp", bufs=2))
        xtp = ctx.enter_context(tc.tile_pool(name="xtp", bufs=9))
        sm = ctx.enter_context(tc.tile_pool(name="sm", bufs=2))
        stat = ctx.enter_context(tc.tile_pool(name="stat", bufs=6))
        ps_p = ctx.enter_context(tc.tile_pool(name="ps_p", bufs=3, space="PSUM"))

        # ---- persistent loads ---------------------------------------
        # x0T as 8 independent tiles on alternating HWDGE queues so the
        # first projection matmul starts as soon as slice 0 lands
        x0T_r = d_x0T.ap().rearrange("(o p) f -> o p f", p=P)
        x0T_tiles = []
        for c in range(NC):
            t = persist.tile([P, T], BF16, tag="x0t", name=f"x0t_{c}",
                             bufs=NC)
            eng = nc.sync if c % 2 == 0 else nc.scalar
            eng.dma_start(t, x0T_r[c])
            x0T_tiles.append(t)

        def x0T_sl(c, fsl):
            return x0T_tiles[c][:, fsl]

        encT_sb = persist.tile([P, NC, S], BF16)
        mask_sb = persist.tile([P, NKS], F32)
        eps_sb = persist.tile([P, 1], F32)
        nc.vector.memset(eps_sb, EPS)
        ident_sb = persist.tile([P, P], BF16)
        from concourse.masks import make_identity
        make_identity(nc, ident_sb)
        if flags["b1"]:
            b1_sb = persist.tile([P, MLP // P], F32)
            nc.sync.dma_start(b1_sb, d_b1.ap())
        rep_sb = {}
        for nm, dt_ in d_rep.items():
            rep_sb[nm] = persist.tile([P, DIM], F32, name="rep_" + nm)
            nc.sync.dma_start(rep_sb[nm], dt_.ap())

        def layer_norm(xr, g_key, b_key):
            st = stat.tile([P, 2, 6], F32, tag="bnst")
            nc.vector.bn_stats(st[:, 0, :], xr[:, 0:512])
            nc.vector.bn_stats(st[:, 1, :], xr[:, 512:1024])
            mv = stat.tile([P, 2], F32, tag="bnmv")
            nc.vector.bn_aggr(mv, st)
            rs = stat.tile([P, 1], F32, tag="rstd")
            nc.scalar.activation(rs, mv[:, 1:2], AF.Sqrt, bias=eps_sb)
            nc.vector.reciprocal(rs, rs)
            nc.vector.tensor_scalar(xr, xr, mv[:, 0:1], rs,
                                    op0=OP.subtract, op1=OP.mult)
            if flags[g_key]:
                nc.vector.tensor_mul(xr, xr, rep_sb[g_key])
            if flags[b_key]:
                nc.vector.tensor_add(xr, xr, rep_sb[b_key])

        x2f = x2T = None
        with ExitStack() as actx:
            wa = actx.enter_context(tc.tile_pool(name="wa", bufs=3))
            qtp = actx.enter_context(tc.tile_pool(name="qtp", bufs=16))
            ktp = actx.enter_context(tc.tile_pool(name="ktp", bufs=8))
            vp = actx.enter_context(tc.tile_pool(name="vp", bufs=9))
            sxp = actx.enter_context(tc.tile_pool(name="sxp", bufs=16))
            ocp = actx.enter_context(tc.tile_pool(name="ocp", bufs=9))
            ps_s = actx.enter_context(
                tc.tile_pool(name="ps_s", bufs=2, space="PSUM"))
            ps_av = actx.enter_context(
                tc.tile_pool(name="ps_av", bufs=2, space="PSUM"))

            # attn weights stream as half-matrices [P, 4, DIM] (8KB/part)
            def load_w(name):
                halves = []
                for hh in range(2):
                    w = wa.tile([P, NC // 2, DIM], BF16, tag="wattn",
                                name=f"{name}_{hh}")
                    eng = nc.sync if hh == 0 else nc.scalar
                    eng.dma_start(
                        w, dw[name].ap().rearrange("(o p) f -> p o f", p=P)
                        [:, ds(4 * hh, 4), :])
                    halves.append(w)
                return lambda c: halves[c // 4][:, c % 4, :]

            def projT(wsel, rhs_fn, nfree, tag):
                outs = []
                for i in range(NI):
                    ps = ps_p.tile([P, nfree], F32, tag="pp", name=f"ps_{tag}{i}")
                    for c in range(NC):
                        nc.tensor.matmul(ps, wsel(c)[:, ts(i, P)], rhs_fn(c),
                                         start=(c == 0), stop=(c == NC - 1))
                    o = qtp.tile([P, nfree], BF16, tag="qt", name=f"{tag}{i}")
                    nc.vector.tensor_copy(o, ps)
                    outs.append(o)
                return outs

            def make_v(wsel, lhsT_fn, nkt, tag):
                vt = []
                for kt in range(nkt):
                    v = vp.tile([P, VW], BF16, tag="v", name=f"{tag}{kt}")
                    ones_ap = v.rearrange("p (h c) -> p h c", c=DH + 1)[
                        :, :, DH:DH + 1]
                    nc.vector.memset(ones_ap, 1.0)
                    for nch in range(2):
                        ps = ps_p.tile([P, 512], F32, tag="pp",
                                       name=f"ps_{tag}{kt}_{nch}")
                        for c in range(NC):
                            nc.tensor.matmul(ps, lhsT_fn(c, kt),
                                             wsel(c)[:, ds(512 * nch, 512)],
                                             start=(c == 0), stop=(c == NC - 1))
                        dst = v.rearrange("p (h c) -> p h c", c=DH + 1)[
                            :, 8 * nch:8 * nch + 8, 0:DH]
                        src = ps.rearrange("p (h c) -> p h c", c=DH)
                        nc.vector.tensor_copy(dst, src)
                    vt.append(v)
                return vt

            def attn_heads(QTt, KTt, Vt, nkt, masked, tag):
                """Head pairs (2p, 2p+1) share inner-tile p at partition
                bases 0/64; their score matmuls are emitted adjacently so
                the PE runs them concurrently in separate row groups."""
                ocat = [ocp.tile([P, TQ], BF16, tag="oc", name=f"{tag}_{i}")
                        for i in range(NI)]
                for p in range(HEADS // 2):
                    exps = ([], [])
                    for kt in range(nkt):
                        for s in range(2):
                            b0 = DH * s
                            pss = ps_s.tile([P, TQ], F32, tag="ps",
                                            name=f"pss_{tag}{p}_{kt}_{s}")
                            nc.tensor.matmul(
                                pss, KTt[p][b0:b0 + DH, ts(kt, P)],
                                QTt[p][b0:b0 + DH, :], start=True, stop=True)
                            e = sxp.tile([P, TQ], BF16, tag="ex",
                                         name=f"ex_{tag}{p}_{kt}_{s}")
                            if masked:
                                nc.scalar.activation(
                                    e, pss, AF.Exp, scale=SCALE,
                                    bias=mask_sb[:, kt:kt + 1])
                            else:
                                nc.scalar.activation(e, pss, AF.Exp,
                                                     scale=SCALE)
                            exps[s].append(e)
                    pavs = [ps_av.tile([DH + 1, TQ], F32, tag="pav",
                                       name=f"pav_{tag}{p}_{s}")
                            for s in range(2)]
                    for kt in range(nkt):
                        for s in range(2):
                            h = 2 * p + s
                            nc.tensor.matmul(
                                pavs[s], Vt[kt][:, ds((DH + 1) * h, DH + 1)],
                                exps[s][kt], start=(kt == 0),
                                stop=(kt == nkt - 1))
                    for s in range(2):
                        pav = pavs[s]
                        rt = sm.tile([DH + 1, TQ], F32, tag="recip",
                                     name=f"rt_{tag}{p}_{s}")
                        nc.vector.reciprocal(rt[DH:DH + 1, :],
                                             pav[DH:DH + 1, :])
                        bt = sm.tile([DH, TQ], F32, tag="bcast",
                                     name=f"bt_{tag}{p}_{s}")
                        nc.gpsimd.dma_start(
                            bt, _bcast_row_ap(rt[DH:DH + 1, :], DH))
                        if s == 0:
                            nc.vector.tensor_mul(ocat[p][0:DH, :],
                                                 pav[0:DH, :], bt)
                        else:
                            ot = sm.tile([DH, TQ], BF16, tag="otmp",
                                         name=f"ot_{tag}{p}")
                            nc.vector.tensor_mul(ot, pav[0:DH, :], bt)
                            nc.gpsimd.dma_start(ocat[p][DH:P, :], ot)
                return ocat

            def outproj_res_ln(ocat, wsel, res_tiles, bias_key, g_key, b_key):
                xres = []
                xT = [xtp.tile([P, TQ], BF16, tag="xT", name=f"xT{b_key}_{j}")
                      for j in range(NC)]
                for mt in range(NMT):
                    xr = xp.tile([P, DIM], F32, tag="xr", name=f"xr{b_key}_{mt}")
                    if res_tiles is None:
                        res = xp.tile([P, DIM], F32, tag="x0r", bufs=1,
                                      name=f"x0r_{mt}")
                        nc.sync.dma_start(res, d_x0h.ap()[ds(P * mt, P), :])
                    else:
                        res = res_tiles[mt]
                    for nch in range(2):
                        ps = ps_p.tile([P, 512], F32, tag="pp",
                                       name=f"psop{b_key}_{mt}_{nch}")
                        for i in range(NI):
                            nc.tensor.matmul(ps, ocat[i][:, ts(mt, P)],
                                             wsel(i)[:, ds(512 * nch, 512)],
                                             start=(i == 0), stop=(i == NI - 1))
                        nc.vector.tensor_add(xr[:, ds(512 * nch, 512)], ps,
                                             res[:, ds(512 * nch, 512)])
                    if flags[bias_key]:
                        nc.vector.tensor_add(xr, xr, rep_sb[bias_key])
                    layer_norm(xr, g_key, b_key)
                    xb = xbp.tile([P, DIM], BF16, tag="xb", name=f"xb{b_key}_{mt}")
                    nc.scalar.activation(xb, xr, AF.Copy)
                    # PE transposes: the PE is otherwise idle at this
                    # boundary, so this is wall-clock free vs DMA transpose
                    for j in range(NC):
                        pst = ps_s.tile([P, P], BF16, tag="ps",
                                        name=f"pst{b_key}_{mt}_{j}")
                        nc.tensor.transpose(pst, xb[:, ts(j, P)], ident_sb)
                        eng = nc.vector if j % 2 == 0 else nc.scalar
                        if j % 2 == 0:
                            nc.vector.tensor_copy(xT[j][:, ts(mt, P)], pst)
                        else:
                            nc.scalar.activation(xT[j][:, ts(mt, P)], pst,
                                                 AF.Copy)
                    xres.append(xr)
                return xres, xT

            # ---- attn1 projections (queries are key-columns 0..TQ) --
            wq1 = load_w("wq1")
            QT1 = projT(wq1, lambda c: x0T_sl(c, slice(0, TQ)), TQ, "qt1")
            wk1 = load_w("wk1")
            KT1 = []
            for i in range(NI):
                k = ktp.tile([P, T], BF16, tag="kt1", name=f"kt1_{i}")
                for nch in range(2):
                    ps = ps_p.tile([P, 512], F32, tag="pp",
                                   name=f"psk1_{i}_{nch}")
                    for c in range(NC):
                        nc.tensor.matmul(ps, wk1(c)[:, ts(i, P)],
                                         x0T_sl(c, ds(512 * nch, 512)),
                                         start=(c == 0), stop=(c == NC - 1))
                    nc.vector.tensor_copy(k[:, ds(512 * nch, 512)], ps)
                KT1.append(k)
            wv1 = load_w("wv1")
            V1 = make_v(wv1, lambda c, kt: x0T_sl(c, ts(kt, P)), NKT, "v1")
            # encoder-side loads are first needed by K2T, emitted here so
            # they don't compete with the startup-critical x0T/wq1 DMAs
            nc.scalar.dma_start(encT_sb,
                                d_encT.ap().rearrange("(o p) f -> p o f", p=P))
            nc.sync.dma_start(mask_sb, d_mask.ap())
            # cross-attn K ready early: PE filler while ACT runs exp
            wk2 = load_w("wk2")
            K2T = projT(wk2, lambda c: encT_sb[:, c, :], S, "k2t")

            # ---- attn1 heads, out-proj, LN1 -------------------------
            # wo1 load emitted BEFORE the heads so its DMA overlaps them
            wo1 = load_w("wo1")
            Ocat1 = attn_heads(QT1, KT1, V1, NKT, masked=False, tag="oc1")
            x1f, x1T = outproj_res_ln(Ocat1, wo1, None, "bo1", "g1", "bb1")

            # cross-attn V (fills the LN1/transpose gap)
            wv2 = load_w("wv2")
            V2 = make_v(wv2, lambda c, kt: encT_sb[:, c, ts(kt, P)], NKS, "v2")

            # ---- attn2 ----------------------------------------------
            wq2 = load_w("wq2")
            QT2 = projT(wq2, lambda c: x1T[c], TQ, "qt2")
            wo2 = load_w("wo2")
            Ocat2 = attn_heads(QT2, K2T, V2, NKS, masked=True, tag="oc2")
            x2f, x2T = outproj_res_ln(Ocat2, wo2, x1f, "bo2", "g2", "bb2")

        # ---- FFN ----------------------------------------------------
        with ExitStack() as fctx:
            ffw1 = fctx.enter_context(tc.tile_pool(name="ffw1", bufs=2))
            ffw2 = fctx.enter_context(tc.tile_pool(name="ffw2", bufs=6))
            htp = fctx.enter_context(tc.tile_pool(name="htp", bufs=32))
            ps_f = fctx.enter_context(
                tc.tile_pool(name="ps_f", bufs=4, space="PSUM"))

            w1r = dw["w1"].ap().rearrange("(o p) f -> p o f", p=P)
            hT = []
            for g in range(8):
                wcb = ffw1.tile([P, NC, 512], BF16, tag="w1cb", name=f"w1cb{g}")
                nc.sync.dma_start(wcb, w1r[:, :, ds(512 * g, 512)])
                for m4 in range(4):
                    ps = ps_p.tile([P, TQ], F32, tag="pp", name=f"psh{g}_{m4}")
                    for c in range(NC):
                        nc.tensor.matmul(ps, wcb[:, c, ts(m4, P)], x2T[c],
                                         start=(c == 0), stop=(c == NC - 1))
                    ht = htp.tile([P, TQ], BF16, tag="ht", name=f"ht{g}_{m4}")
                    b1b = (b1_sb[:, 4 * g + m4:4 * g + m4 + 1]
                           if flags["b1"] else 0.0)
                    if sim_gelu:
                        # CoreSim lacks Gelu; sigmoid-approx for sim runs only
                        sg = htp.tile([P, TQ], F32, tag="sg", bufs=2,
                                      name=f"sg{g}_{m4}")
                        nc.scalar.activation(sg, ps, AF.Sigmoid, scale=1.702,
                                             bias=b1b)
                        nc.vector.tensor_mul(ht, ps, sg)
                    else:
                        nc.scalar.activation(ht, ps, AF.Gelu, bias=b1b)
                    hT.append(ht)

            # ffn2 in mt groups: each group's epilogue (residual, LN3,
            # output DMA) overlaps the next group's matmuls; the final
            # group is a single tile so the serial tail is minimal
            for half, mts in enumerate([(0, 1), (2, 3)]):
                psy = {(mt, nch): ps_f.tile([P, 512], F32, tag="pf",
                                            name=f"psy{half}_{mt}_{nch}")
                       for mt in mts for nch in range(2)}
                for kt in range(MLP // P):
                    w2s = ffw2.tile([P, DIM], BF16, tag="w2s",
                                    name=f"w2s{half}_{kt}")
                    eng = nc.sync if kt % 2 == 0 else nc.scalar
                    eng.dma_start(w2s, dw["w2"].ap()[ds(P * kt, P), :])
                    for mt in mts:
                        for nch in range(2):
                            nc.tensor.matmul(psy[(mt, nch)],
                                             hT[kt][:, ts(mt, P)],
                                             w2s[:, ds(512 * nch, 512)],
                                             start=(kt == 0),
                                             stop=(kt == MLP // P - 1))
                for mt in mts:
                    xr = xp.tile([P, DIM], F32, tag="xr", name=f"x3r_{mt}")
                    for nch in range(2):
                        nc.vector.tensor_add(xr[:, ds(512 * nch, 512)],
                                             psy[(mt, nch)],
                                             x2f[mt][:, ds(512 * nch, 512)])
                    if flags["b2"]:
                        nc.vector.tensor_add(xr, xr, rep_sb["b2"])
                    layer_norm(xr, "g3", "bb3")
                    eng = nc.sync if mt % 2 == 0 else nc.scalar
                    eng.dma_start(d_out.ap()[ds(P * mt, P), :], xr)

    nc.compile()
    return nc


def _host_prep(inputs):
    """Numpy-side sharding and packing. Returns (flags, in_maps)."""
    f32 = np.float32
    bf16 = ml_dtypes.bfloat16
    dec = np.asarray(inputs["dec_input"], f32)        # [4, 1024]
    enc = np.asarray(inputs["enc_output"], f32)       # [4, 512, 1024]
    enc_in = np.asarray(inputs["enc_input"], f32)     # [4, 511]
    pe_w = np.asarray(inputs["pe_w"], f32)
    pe_b = np.asarray(inputs["pe_b"], f32)
    pos = np.asarray(inputs["pos_emb"], f32)
    lp = inputs["params"][-1]                         # only the last layer matters
    a1, a2, ff = lp["a1"], lp["a2"], lp["ff"]

    W = {
        "wq1": a1["wq"], "wk1": a1["wk"], "wv1": a1["wv"], "wo1": a1["wo"],
        "wq2": a2["wq"], "wk2": a2["wk"], "wv2": a2["wv"], "wo2": a2["wo"],
        "w1": ff["w1"], "w2": ff["w2"],
    }
    W = {k: np.ascontiguousarray(np.asarray(v, f32).astype(bf16))
         for k, v in W.items()}

    def vec(x):
        return np.asarray(x, f32).reshape(-1)

    bo1, bo2 = vec(a1["bo"]), vec(a2["bo"])
    b1v, b2v = vec(ff["b1"]), vec(ff["b2"])
    g1, bb1 = vec(a1["g"]), vec(a1["b"])
    g2, bb2 = vec(a2["g"]), vec(a2["b"])
    g3, bb3 = vec(ff["g"]), vec(ff["b"])
    flags = {
        "bo1": bool(np.any(bo1 != 0)), "bo2": bool(np.any(bo2 != 0)),
        "b1": bool(np.any(b1v != 0)), "b2": bool(np.any(b2v != 0)),
        "g1": bool(np.any(g1 != 1)), "bb1": bool(np.any(bb1 != 0)),
        "g2": bool(np.any(g2 != 1)), "bb2": bool(np.any(bb2 != 0)),
        "g3": bool(np.any(g3 != 1)), "bb3": bool(np.any(bb3 != 0)),
    }

    shared = dict(W)
    if flags["b1"]:
        shared["b1pk"] = np.ascontiguousarray(
            b1v.reshape(MLP // P, P).T.astype(f32))
    reps = {"bo1": bo1, "bo2": bo2, "b2": b2v, "g1": g1, "bb1": bb1,
            "g2": g2, "bb2": bb2, "g3": g3, "bb3": bb3}
    for nm, v in reps.items():
        if flags[nm]:
            shared["rep_" + nm] = np.ascontiguousarray(
                np.broadcast_to(v[None, :], (P, DIM)).astype(f32))

    # x0 = dec[..., None] * pe_w + pe_b + pos  (matches reference f32 order)
    x0 = (dec[..., None] * pe_w + pe_b) + pos         # [4, 1024, 1024]
    x0 = np.asarray(x0, f32)

    # cross-attn additive mask bias per batch: keys [1 | enc_input]
    enc_cat = np.concatenate([np.ones((4, 1), f32), enc_in], axis=1)  # [4,512]
    mbias = np.where(enc_cat == 0.0, f32(-1e9), f32(0.0)).astype(f32)

    in_maps = []
    for core in range(8):
        b, h = core // 2, core % 2
        x0b = x0[b]                                   # [1024, 1024]
        # rotate tokens so own queries are key-columns 0..TQ-1
        order = np.r_[h * TQ:(h + 1) * TQ, (1 - h) * TQ:(2 - h) * TQ]
        x0Tb = np.ascontiguousarray(x0b[order].T.astype(bf16))
        m = {
            "x0T": x0Tb,
            "x0h": np.ascontiguousarray(x0b[h * TQ:(h + 1) * TQ]),
            "encT": np.ascontiguousarray(enc[b].T.astype(bf16)),
            "maskb": np.ascontiguousarray(mbias[b].reshape(NKS, P).T),
        }
        m.update(shared)
        in_maps.append(m)
    return flags, in_maps


_CACHE = {}
LAST_RESULTS = None


def kernel(**inputs) -> np.ndarray:
    global LAST_RESULTS
    flags, in_maps = _host_prep(inputs)
    key = tuple(sorted(flags.items()))
    if key not in _CACHE:
        _CACHE[key] = build_program(flags)
    nc = _CACHE[key]
    res = run_bass_kernel_spmd(nc, in_maps, core_ids=list(range(8)))
    LAST_RESULTS = res
    out = np.empty((4, T, DIM), np.float32)
    for core in range(8):
        b, h = core // 2, core % 2
        out[b, h * TQ:(h + 1) * TQ, :] = res.results[core]["outp"]
    return out


# revision 40
# speedup vs baseline: 12940.9934x; 11251.4368x over previous
"""Trainium2 Bass kernel for the nn_Decoder problem.

Contract: kernel(**inputs) takes the FULL (unsharded) numpy inputs and
returns the FULL output [4, 1024, 1024] f32.

Sharding: (batch b, sequence-half h) -> 8 cores, zero collectives.
Key structural fact about the reference: inside the depth loop, x is
overwritten by `_ln(_attn(x0,x0,x0,a1) + x0, ...)` which does not read
the previous iteration's x, so only the LAST layer's block contributes
to the output. The kernel therefore computes a single layer (params[-1]).

SPMD trick: the program is identical on all cores; the host rotates the
x0^T key columns per core so that the core's own 512 query tokens are
always columns 0..511 (attention is permutation-invariant over keys).
"""

import os
import sys
from contextlib import ExitStack

for _p in ("/opt/trn_rl_repo", "/root/.axon_site/_ro/trn_rl_repo"):
    if os.path.isdir(_p) and _p not in sys.path:
        sys.path.insert(0, _p)

import numpy as np
import ml_dtypes

import concourse.bass as bass
import concourse.tile as tile
from concourse import bacc, mybir
from concourse.bass import ds, ts
from concourse.bass_utils import run_bass_kernel_spmd

BF16 = mybir.dt.bfloat16
F32 = mybir.dt.float32
AF = mybir.ActivationFunctionType
OP = mybir.AluOpType

P = 128
DIM = 1024          # model dim (8 tiles of 128)
HEADS = 16
DH = 64             # head dim
INNER = HEADS * DH  # 1024
MLP = 4096          # 32 tiles of 128, 8 column-blocks of 512
T = 1024            # full decoder sequence (8 key tiles)
TQ = 512            # tokens per core (4 tile rows)
S = 512             # encoder sequence (4 key tiles)
SCALE = DH ** -0.5
EPS = 1e-5
NC = DIM // P       # 8  contraction tiles over model dim
NI = INNER // P     # 8  inner-dim tiles (2 heads each)
NKT = T // P        # 8  self-attn key tiles
NKS = S // P        # 4  cross-attn key tiles
NMT = TQ // P       # 4  own-token tiles
VW = HEADS * (DH + 1)  # 1040: V tiles carry a ones column per head


def _bcast_row_ap(row_ap, nparts):
    """Broadcast a [1, N] SBUF row for DMA: keep the (nonzero-step)
    partition dim at count 1 and replicate via a 0-step free dim."""
    return bass.AP(
        tensor=row_ap.tensor,
        offset=row_ap.offset,
        ap=[list(row_ap.ap[0]), [0, nparts], list(row_ap.ap[-1])],
    )


def build_program(flags, sim_gelu=False):
    """Build the single-core SPMD program. flags: dict of bias/affine gates."""
    nc = bacc.Bacc("TRN2", target_bir_lowering=False, debug=False)

    # ---- DRAM I/O ----------------------------------------------------
    d_x0T = nc.dram_tensor("x0T", [DIM, T], BF16, kind="ExternalInput")
    d_x0h = nc.dram_tensor("x0h", [TQ, DIM], F32, kind="ExternalInput")
    d_encT = nc.dram_tensor("encT", [DIM, S], BF16, kind="ExternalInput")
    d_mask = nc.dram_tensor("maskb", [P, NKS], F32, kind="ExternalInput")
    dw = {}
    for nm in ("wq1", "wk1", "wv1", "wo1", "wq2", "wk2", "wv2", "wo2"):
        dw[nm] = nc.dram_tensor(nm, [DIM, DIM], BF16, kind="ExternalInput")
    dw["w1"] = nc.dram_tensor("w1", [DIM, MLP], BF16, kind="ExternalInput")
    dw["w2"] = nc.dram_tensor("w2", [MLP, DIM], BF16, kind="ExternalInput")
    if flags["b1"]:
        d_b1 = nc.dram_tensor("b1pk", [P, MLP // P], F32, kind="ExternalInput")
    d_rep = {}
    for nm in ("bo1", "bo2", "b2", "g1", "bb1", "g2", "bb2", "g3", "bb3"):
        if flags[nm]:
            d_rep[nm] = nc.dram_tensor("rep_" + nm, [P, DIM], F32,
                                       kind="ExternalInput")
    d_out = nc.dram_tensor("outp", [TQ, DIM], F32, kind="ExternalOutput")

    with tile.TileContext(nc) as tc, ExitStack() as ctx:
        # ---- whole-kernel pools (per-partition KB is the scarce axis) -
        persist = ctx.enter_context(tc.tile_pool(name="persist", bufs=1))
        xp = ctx.enter_context(tc.tile_pool(name="xp", bufs=8))
        xbp = ctx.enter_context(tc.tile_pool(name="xbp", bufs=2))
        xtp = ctx.enter_context(tc.tile_pool(name="xtp", bufs=9))
        sm = ctx.enter_context(tc.tile_pool(name="sm", bufs=2))
        stat = ctx.enter_context(tc.tile_pool(name="stat", bufs=6))
        ps_p = ctx.enter_context(tc.tile_pool(name="ps_p", bufs=3, space="PSUM"))

        # ---- persistent loads ---------------------------------------
        # x0T as 8 independent tiles on alternating HWDGE queues so the
        # first projection matmul starts as soon as slice 0 lands
        x0T_r = d_x0T.ap().rearrange("(o p) f -> o p f", p=P)
        x0T_tiles = []
        for c in range(NC):
            t = persist.tile([P, T], BF16, tag="x0t", name=f"x0t_{c}",
                             bufs=NC)
            eng = nc.sync if c % 2 == 0 else nc.scalar
            eng.dma_start(t, x0T_r[c])
            x0T_tiles.append(t)

        def x0T_sl(c, fsl):
            return x0T_tiles[c][:, fsl]

        encT_sb = persist.tile([P, NC, S], BF16)
        mask_sb = persist.tile([P, NKS], F32)
        eps_sb = persist.tile([P, 1], F32)
        nc.vector.memset(eps_sb, EPS)
        ident_sb = persist.tile([P, P], BF16)
        from concourse.masks import make_identity
        make_identity(nc, ident_sb)
        if flags["b1"]:
            b1_sb = persist.tile([P, MLP // P], F32)
            nc.sync.dma_start(b1_sb, d_b1.ap())
        rep_sb = {}
        for nm, dt_ in d_rep.items():
            rep_sb[nm] = persist.tile([P, DIM], F32, name="rep_" + nm)
            nc.sync.dma_start(rep_sb[nm], dt_.ap())

        def layer_norm(xr, g_key, b_key):
            st = stat.tile([P, 2, 6], F32, tag="bnst")
            nc.vector.bn_stats(st[:, 0, :], xr[:, 0:512])
            nc.vector.bn_stats(st[:, 1, :], xr[:, 512:1024])
            mv = stat.tile([P, 2], F32, tag="bnmv")
            nc.vector.bn_aggr(mv, st)
            rs = stat.tile([P, 1], F32, tag="rstd")
            nc.scalar.activation(rs, mv[:, 1:2], AF.Sqrt, bias=eps_sb)
            nc.vector.reciprocal(rs, rs)
            nc.vector.tensor_scalar(xr, xr, mv[:, 0:1], rs,
                                    op0=OP.subtract, op1=OP.mult)
            if flags[g_key]:
                nc.vector.tensor_mul(xr, xr, rep_sb[g_key])
            if flags[b_key]:
                nc.vector.tensor_add(xr, xr, rep_sb[b_key])

        x2f = x2T = None
        with ExitStack() as actx:
            wa = actx.enter_context(tc.tile_pool(name="wa", bufs=8))
            qtp = actx.enter_context(tc.tile_pool(name="qtp", bufs=16))
            ktp = actx.enter_context(tc.tile_pool(name="ktp", bufs=8))
            vp = actx.enter_context(tc.tile_pool(name="vp", bufs=13))
            sxp = actx.enter_context(tc.tile_pool(name="sxp", bufs=16))
            ocp = actx.enter_context(tc.tile_pool(name="ocp", bufs=9))
            ps_s = actx.enter_context(
                tc.tile_pool(name="ps_s", bufs=3, space="PSUM"))
            ps_av = actx.enter_context(
                tc.tile_pool(name="ps_av", bufs=2, space="PSUM"))

            # attn weights stream as quarter-matrices [P, 2, DIM]
            # (4KB/part): finer DMA granularity so the first matmuls of a
            # projection start ~1.5us after the weight DMA kicks off
            def load_w(name):
                quarters = []
                for qq in range(4):
                    w = wa.tile([P, NC // 4, DIM], BF16, tag="wattn",
                                name=f"{name}_{qq}")
                    eng = nc.sync if qq % 2 == 0 else nc.scalar
                    eng.dma_start(
                        w, dw[name].ap().rearrange("(o p) f -> p o f", p=P)
                        [:, ds(2 * qq, 2), :])
                    quarters.append(w)
                return lambda c: quarters[c // 2][:, c % 2, :]

            def projT(wsel, rhs_fn, nfree, tag):
                outs = []
                for i in range(NI):
                    ps = ps_p.tile([P, nfree], F32, tag="pp", name=f"ps_{tag}{i}")
                    for c in range(NC):
                        nc.tensor.matmul(ps, wsel(c)[:, ts(i, P)], rhs_fn(c),
                                         start=(c == 0), stop=(c == NC - 1))
                    o = qtp.tile([P, nfree], BF16, tag="qt", name=f"{tag}{i}")
                    nc.vector.tensor_copy(o, ps)
                    outs.append(o)
                return outs

            def make_v(wsel, lhsT_fn, nkt, tag, pspool=None, pstag="pp"):
                pspool = pspool or ps_p
                vt = []
                for kt in range(nkt):
                    v = vp.tile([P, VW], BF16, tag="v", name=f"{tag}{kt}")
                    ones_ap = v.rearrange("p (h c) -> p h c", c=DH + 1)[
                        :, :, DH:DH + 1]
                    nc.vector.memset(ones_ap, 1.0)
                    for nch in range(2):
                        ps = pspool.tile([P, 512], F32, tag=pstag,
                                         name=f"ps_{tag}{kt}_{nch}")
                        for c in range(NC):
                            nc.tensor.matmul(ps, lhsT_fn(c, kt),
                                             wsel(c)[:, ds(512 * nch, 512)],
                                             start=(c == 0), stop=(c == NC - 1))
                        dst = v.rearrange("p (h c) -> p h c", c=DH + 1)[
                            :, 8 * nch:8 * nch + 8, 0:DH]
                        src = ps.rearrange("p (h c) -> p h c", c=DH)
                        nc.vector.tensor_copy(dst, src)
                    vt.append(v)
                return vt

            def attn_heads(QTt, KTt, Vt, nkt, masked, tag, fillers=None):
                """Head pairs (2p, 2p+1) share inner-tile p at partition
                bases 0/64; their score matmuls are emitted adjacently so
                the PE runs them concurrently in separate row groups.
                fillers[p] emits independent PE work after pair p so the
                scheduler can cover the ACT-bound stretches."""
                ocat = [ocp.tile([P, TQ], BF16, tag="oc", name=f"{tag}_{i}")
                        for i in range(NI)]
                for p in range(HEADS // 2):
                    exps = ([], [])
                    for kt in range(nkt):
                        for s in range(2):
                            b0 = DH * s
                            pss = ps_s.tile([P, TQ], F32, tag="ps",
                                            name=f"pss_{tag}{p}_{kt}_{s}")
                            nc.tensor.matmul(
                                pss, KTt[p][b0:b0 + DH, ts(kt, P)],
                                QTt[p][b0:b0 + DH, :], start=True, stop=True)
                            e = sxp.tile([P, TQ], BF16, tag="ex",
                                         name=f"ex_{tag}{p}_{kt}_{s}")
                            if masked:
                                nc.scalar.activation(
                                    e, pss, AF.Exp, scale=SCALE,
                                    bias=mask_sb[:, kt:kt + 1])
                            else:
                                nc.scalar.activation(e, pss, AF.Exp,
                                                     scale=SCALE)
                            exps[s].append(e)
                    pavs = [ps_av.tile([DH + 1, TQ], F32, tag="pav",
                                       name=f"pav_{tag}{p}_{s}")
                            for s in range(2)]
                    for kt in range(nkt):
                        for s in range(2):
                            h = 2 * p + s
                            nc.tensor.matmul(
                                pavs[s], Vt[kt][:, ds((DH + 1) * h, DH + 1)],
                                exps[s][kt], start=(kt == 0),
                                stop=(kt == nkt - 1))
                    for s in range(2):
                        pav = pavs[s]
                        rt = sm.tile([DH + 1, TQ], F32, tag="recip",
                                     name=f"rt_{tag}{p}_{s}")
                        nc.vector.reciprocal(rt[DH:DH + 1, :],
                                             pav[DH:DH + 1, :])
                        bt = sm.tile([DH, TQ], F32, tag="bcast",
                                     name=f"bt_{tag}{p}_{s}")
                        # SWDGE (gpsimd): HWDGE wedges on 0-step-dim APs
                        nc.gpsimd.dma_start(
                            bt, _bcast_row_ap(rt[DH:DH + 1, :], DH))
                        if s == 0:
                            nc.vector.tensor_mul(ocat[p][0:DH, :],
                                                 pav[0:DH, :], bt)
                        else:
                            ot = sm.tile([DH, TQ], BF16, tag="otmp",
                                         name=f"ot_{tag}{p}")
                            nc.vector.tensor_mul(ot, pav[0:DH, :], bt)
                            nc.gpsimd.dma_start(ocat[p][DH:P, :], ot)
                    if fillers is not None:
                        fillers[p]()
                return ocat

            def outproj_res_ln(ocat, wsel, res_tiles, bias_key, g_key, b_key):
                xres = []
                xT = [xtp.tile([P, TQ], BF16, tag="xT", name=f"xT{b_key}_{j}")
                      for j in range(NC)]
                for mt in range(NMT):
                    xr = xp.tile([P, DIM], F32, tag="xr", name=f"xr{b_key}_{mt}")
                    if res_tiles is None:
                        res = xp.tile([P, DIM], F32, tag="x0r", bufs=1,
                                      name=f"x0r_{mt}")
                        nc.sync.dma_start(res, d_x0h.ap()[ds(P * mt, P), :])
                    else:
                        res = res_tiles[mt]
                    for nch in range(2):
                        ps = ps_p.tile([P, 512], F32, tag="pp",
                                       name=f"psop{b_key}_{mt}_{nch}")
                        for i in range(NI):
                            nc.tensor.matmul(ps, ocat[i][:, ts(mt, P)],
                                             wsel(i)[:, ds(512 * nch, 512)],
                                             start=(i == 0), stop=(i == NI - 1))
                        nc.vector.tensor_add(xr[:, ds(512 * nch, 512)], ps,
                                             res[:, ds(512 * nch, 512)])
                    if flags[bias_key]:
                        nc.vector.tensor_add(xr, xr, rep_sb[bias_key])
                    layer_norm(xr, g_key, b_key)
                    xb = xbp.tile([P, DIM], BF16, tag="xb", name=f"xb{b_key}_{mt}")
                    nc.scalar.activation(xb, xr, AF.Copy)
                    # PE transposes: the PE is otherwise idle at this
                    # boundary, so this is wall-clock free vs DMA transpose
                    for j in range(NC):
                        pst = ps_s.tile([P, P], BF16, tag="ps",
                                        name=f"pst{b_key}_{mt}_{j}")
                        nc.tensor.transpose(pst, xb[:, ts(j, P)], ident_sb)
                        eng = nc.vector if j % 2 == 0 else nc.scalar
                        if j % 2 == 0:
                            nc.vector.tensor_copy(xT[j][:, ts(mt, P)], pst)
                        else:
                            nc.scalar.activation(xT[j][:, ts(mt, P)], pst,
                                                 AF.Copy)
                    xres.append(xr)
                return xres, xT

            # ---- attn1 projections (queries are key-columns 0..TQ) --
            wq1 = load_w("wq1")
            QT1 = projT(wq1, lambda c: x0T_sl(c, slice(0, TQ)), TQ, "qt1")
            wk1 = load_w("wk1")
            KT1 = []
            for i in range(NI):
                k = ktp.tile([P, T], BF16, tag="kt1", name=f"kt1_{i}")
                for nch in range(2):
                    ps = ps_p.tile([P, 512], F32, tag="pp",
                                   name=f"psk1_{i}_{nch}")
                    for c in range(NC):
                        nc.tensor.matmul(ps, wk1(c)[:, ts(i, P)],
                                         x0T_sl(c, ds(512 * nch, 512)),
                                         start=(c == 0), stop=(c == NC - 1))
                    nc.vector.tensor_copy(k[:, ds(512 * nch, 512)], ps)
                KT1.append(k)
            wv1 = load_w("wv1")
            V1 = make_v(wv1, lambda c, kt: x0T_sl(c, ts(kt, P)), NKT, "v1")
            # encoder-side loads are first needed by K2T, emitted here so
            # they don't compete with the startup-critical x0T/wq1 DMAs
            nc.scalar.dma_start(encT_sb,
                                d_encT.ap().rearrange("(o p) f -> p o f", p=P))
            nc.sync.dma_start(mask_sb, d_mask.ap())
            wk2 = load_w("wk2")
            wv2 = load_w("wv2")
            # wo1 load emitted BEFORE the heads so its DMA overlaps them
            wo1 = load_w("wo1")

            # cross-attn K2T / V2 are independent of attn1; emit one block
            # after each attn1 head pair so the scheduler can fill the
            # ACT-bound stretches of the head phase with their matmuls
            K2T = [None] * NI
            V2 = [None] * NKS

            def mk_filler(p):
                def _fill():
                    # K2T block i=p
                    ps = ps_p.tile([P, S], F32, tag="pp", name=f"ps_k2t{p}")
                    for c in range(NC):
                        nc.tensor.matmul(ps, wk2(c)[:, ts(p, P)],
                                         encT_sb[:, c, :],
                                         start=(c == 0), stop=(c == NC - 1))
                    o = qtp.tile([P, S], BF16, tag="qt", name=f"k2t{p}")
                    nc.vector.tensor_copy(o, ps)
                    K2T[p] = o
                    # V2 chunk (kt, nch) = (p//2, p%2)
                    kt, nch = p // 2, p % 2
                    if nch == 0:
                        v = vp.tile([P, VW], BF16, tag="v", name=f"v2{kt}")
                        ones_ap = v.rearrange("p (h c) -> p h c", c=DH + 1)[
                            :, :, DH:DH + 1]
                        nc.vector.memset(ones_ap, 1.0)
                        V2[kt] = v
                    v = V2[kt]
                    ps2 = ps_p.tile([P, 512], F32, tag="pp",
                                    name=f"ps_v2{kt}_{nch}")
                    for c in range(NC):
                        nc.tensor.matmul(ps2, encT_sb[:, c, ts(kt, P)],
                                         wv2(c)[:, ds(512 * nch, 512)],
                                         start=(c == 0), stop=(c == NC - 1))
                    dst = v.rearrange("p (h c) -> p h c", c=DH + 1)[
                        :, 8 * nch:8 * nch + 8, 0:DH]
                    nc.vector.tensor_copy(
                        dst, ps2.rearrange("p (h c) -> p h c", c=DH))
                return _fill

            # ---- attn1 heads, out-proj, LN1 -------------------------
            Ocat1 = attn_heads(QT1, KT1, V1, NKT, masked=False, tag="oc1",
                               fillers=[mk_filler(p) for p in range(8)])
            x1f, x1T = outproj_res_ln(Ocat1, wo1, None, "bo1", "g1", "bb1")

            # ---- attn2 ----------------------------------------------
            wq2 = load_w("wq2")
            QT2 = projT(wq2, lambda c: x1T[c], TQ, "qt2")
            wo2 = load_w("wo2")
            Ocat2 = attn_heads(QT2, K2T, V2, NKS, masked=True, tag="oc2")
            x2f, x2T = outproj_res_ln(Ocat2, wo2, x1f, "bo2", "g2", "bb2")

        # ---- FFN ----------------------------------------------------
        with ExitStack() as fctx:
            ffw1 = fctx.enter_context(tc.tile_pool(name="ffw1", bufs=2))
            ffw2 = fctx.enter_context(tc.tile_pool(name="ffw2", bufs=6))
            htp = fctx.enter_context(tc.tile_pool(name="htp", bufs=32))
            ps_f = fctx.enter_context(
                tc.tile_pool(name="ps_f", bufs=4, space="PSUM"))

            w1r = dw["w1"].ap().rearrange("(o p) f -> p o f", p=P)
            hT = []
            for g in range(8):
                wcb = ffw1.tile([P, NC, 512], BF16, tag="w1cb", name=f"w1cb{g}")
                nc.sync.dma_start(wcb, w1r[:, :, ds(512 * g, 512)])
                if g == 0:
                    # first block: c-outer over 4 held PSUMs so matmuls
                    # pipeline with the x2T transposes as they land
                    psg = [ps_f.tile([P, TQ], F32, tag="pf",
                                     name=f"psh0_{m4}") for m4 in range(4)]
                    for c in range(NC):
                        for m4 in range(4):
                            nc.tensor.matmul(psg[m4], wcb[:, c, ts(m4, P)],
                                             x2T[c], start=(c == 0),
                                             stop=(c == NC - 1))
                    for m4 in range(4):
                        ht = htp.tile([P, TQ], BF16, tag="ht",
                                      name=f"ht0_{m4}")
                        b1b = (b1_sb[:, m4:m4 + 1] if flags["b1"] else 0.0)
                        if sim_gelu:
                            sg = htp.tile([P, TQ], F32, tag="sg", bufs=2,
                                          name=f"sg0_{m4}")
                            nc.scalar.activation(sg, psg[m4], AF.Sigmoid,
                                                 scale=1.702, bias=b1b)
                            nc.vector.tensor_mul(ht, psg[m4], sg)
                        else:
                            nc.scalar.activation(ht, psg[m4], AF.Gelu,
                                                 bias=b1b)
                        hT.append(ht)
                    continue
                for m4 in range(4):
                    ps = ps_p.tile([P, TQ], F32, tag="pp", name=f"psh{g}_{m4}")
                    for c in range(NC):
                        nc.tensor.matmul(ps, wcb[:, c, ts(m4, P)], x2T[c],
                                         start=(c == 0), stop=(c == NC - 1))
                    ht = htp.tile([P, TQ], BF16, tag="ht", name=f"ht{g}_{m4}")
                    b1b = (b1_sb[:, 4 * g + m4:4 * g + m4 + 1]
                           if flags["b1"] else 0.0)
                    if sim_gelu:
                        # CoreSim lacks Gelu; sigmoid-approx for sim runs only
                        sg = htp.tile([P, TQ], F32, tag="sg", bufs=2,
                                      name=f"sg{g}_{m4}")
                        nc.scalar.activation(sg, ps, AF.Sigmoid, scale=1.702,
                                             bias=b1b)
                        nc.vector.tensor_mul(ht, ps, sg)
                    else:
                        nc.scalar.activation(ht, ps, AF.Gelu, bias=b1b)
                    hT.append(ht)

            # ffn2 in mt groups: each group's epilogue (residual, LN3,
            # output DMA) overlaps the next group's matmuls; the final
            # group is a single tile so the serial tail is minimal
            for half, mts in enumerate([(0, 1), (2, 3)]):
                psy = {(mt, nch): ps_f.tile([P, 512], F32, tag="pf",
                                            name=f"psy{half}_{mt}_{nch}")
                       for mt in mts for nch in range(2)}
                for kt in range(MLP // P):
                    w2s = ffw2.tile([P, DIM], BF16, tag="w2s",
                                    name=f"w2s{half}_{kt}")
                    eng = nc.sync if kt % 2 == 0 else nc.scalar
                    eng.dma_start(w2s, dw["w2"].ap()[ds(P * kt, P), :])
                    for mt in mts:
                        for nch in range(2):
                            nc.tensor.matmul(psy[(mt, nch)],
                                             hT[kt][:, ts(mt, P)],
                                             w2s[:, ds(512 * nch, 512)],
                                             start=(kt == 0),
                                             stop=(kt == MLP // P - 1))
                for mt in mts:
                    xr = xp.tile([P, DIM], F32, tag="xr", name=f"x3r_{mt}")
                    for nch in range(2):
                        nc.vector.tensor_add(xr[:, ds(512 * nch, 512)],
                                             psy[(mt, nch)],
                                             x2f[mt][:, ds(512 * nch, 512)])
                    if flags["b2"]:
                        nc.vector.tensor_add(xr, xr, rep_sb["b2"])
                    layer_norm(xr, "g3", "bb3")
                    eng = nc.sync if mt % 2 == 0 else nc.scalar
                    eng.dma_start(d_out.ap()[ds(P * mt, P), :], xr)

    nc.compile()
    return nc


def _host_prep(inputs):
    """Numpy-side sharding and packing. Returns (flags, in_maps)."""
    f32 = np.float32
    bf16 = ml_dtypes.bfloat16
    dec = np.asarray(inputs["dec_input"], f32)        # [4, 1024]
    enc = np.asarray(inputs["enc_output"], f32)       # [4, 512, 1024]
    enc_in = np.asarray(inputs["enc_input"], f32)     # [4, 511]
    pe_w = np.asarray(inputs["pe_w"], f32)
    pe_b = np.asarray(inputs["pe_b"], f32)
    pos = np.asarray(inputs["pos_emb"], f32)
    lp = inputs["params"][-1]                         # only the last layer matters
    a1, a2, ff = lp["a1"], lp["a2"], lp["ff"]

    W = {
        "wq1": a1["wq"], "wk1": a1["wk"], "wv1": a1["wv"], "wo1": a1["wo"],
        "wq2": a2["wq"], "wk2": a2["wk"], "wv2": a2["wv"], "wo2": a2["wo"],
        "w1": ff["w1"], "w2": ff["w2"],
    }
    W = {k: np.ascontiguousarray(np.asarray(v, f32).astype(bf16))
         for k, v in W.items()}

    def vec(x):
        return np.asarray(x, f32).reshape(-1)

    bo1, bo2 = vec(a1["bo"]), vec(a2["bo"])
    b1v, b2v = vec(ff["b1"]), vec(ff["b2"])
    g1, bb1 = vec(a1["g"]), vec(a1["b"])
    g2, bb2 = vec(a2["g"]), vec(a2["b"])
    g3, bb3 = vec(ff["g"]), vec(ff["b"])
    flags = {
        "bo1": bool(np.any(bo1 != 0)), "bo2": bool(np.any(bo2 != 0)),
        "b1": bool(np.any(b1v != 0)), "b2": bool(np.any(b2v != 0)),
        "g1": bool(np.any(g1 != 1)), "bb1": bool(np.any(bb1 != 0)),
        "g2": bool(np.any(g2 != 1)), "bb2": bool(np.any(bb2 != 0)),
        "g3": bool(np.any(g3 != 1)), "bb3": bool(np.any(bb3 != 0)),
    }

    shared = dict(W)
    if flags["b1"]:
        shared["b1pk"] = np.ascontiguousarray(
            b1v.reshape(MLP // P, P).T.astype(f32))
    reps = {"bo1": bo1, "bo2": bo2, "b2": b2v, "g1": g1, "bb1": bb1,
            "g2": g2, "bb2": bb2, "g3": g3, "bb3": bb3}
    for nm, v in reps.items():
        if flags[nm]:
            shared["rep_" + nm] = np.ascontiguousarray(
                np.broadcast_to(v[None, :], (P, DIM)).astype(f32))

    # x0 = dec[..., None] * pe_w + pe_b + pos  (matches reference f32 order)
    x0 = (dec[..., None] * pe_w + pe_b) + pos         # [4, 1024, 1024]
    x0 = np.asarray(x0, f32)

    # cross-attn additive mask bias per batch: keys [1 | enc_input]
    enc_cat = np.concatenate([np.ones((4, 1), f32), enc_in], axis=1)  # [4,512]
    mbias = np.where(enc_cat == 0.0, f32(-1e9), f32(0.0)).astype(f32)

    in_maps = []
    for core in range(8):
        b, h = core // 2, core % 2
        x0b = x0[b]                                   # [1024, 1024]
        # rotate tokens so own queries are key-columns 0..TQ-1
        order = np.r_[h * TQ:(h + 1) * TQ, (1 - h) * TQ:(2 - h) * TQ]
        x0Tb = np.ascontiguousarray(x0b[order].T.astype(bf16))
        m = {
            "x0T": x0Tb,
            "x0h": np.ascontiguousarray(x0b[h * TQ:(h + 1) * TQ]),
            "encT": np.ascontiguousarray(enc[b].T.astype(bf16)),
            "maskb": np.ascontiguousarray(mbias[b].reshape(NKS, P).T),
        }
        m.update(shared)
        in_maps.append(m)
    return flags, in_maps


_CACHE = {}
LAST_RESULTS = None


def kernel(**inputs) -> np.ndarray:
    global LAST_RESULTS
    flags, in_maps = _host_prep(inputs)
    key = tuple(sorted(flags.items()))
    if key not in _CACHE:
        _CACHE[key] = build_program(flags)
    nc = _CACHE[key]
    res = run_bass_kernel_spmd(nc, in_maps, core_ids=list(range(8)))
    LAST_RESULTS = res
    out = np.empty((4, T, DIM), np.float32)
    for core in range(8):
        b, h = core // 2, core % 2
        out[b, h * TQ:(h + 1) * TQ, :] = res.results[core]["outp"]
    return out
